# revision 33
# baseline (speedup 1.0000x reference)
"""Trainium2 Bass kernel for nn_MoEConnectionProcessor (v3: fp8 DoubleRow chains).

Strategy (delta over v2)
------------------------
Data-parallel over 8 cores; per core 16 super-tiles (ST) of 2048 cells in
blockT layout: SBUF partition = (g, d) (cell-subgroup x feature), free
axis = (t, c) = 512 cols per ST.

v3 structural changes (415us -> ~267us measured on HW):
  * The three premasked neighbor copies are staged as ONE concatenated
    fp8e4m3 tensor [tier0 | tier1 | tier2] with per-band widths padded to
    EVEN.  m0/m2 use host-side error-feedback quantization (per-cell carry
    across slots) so the tier SUMS keep near-bf16 accuracy.
  * All slot sums run on the PE as DoubleRow fp8 accumulation chains
    (2 slots per matmul, 2x rate) with a single shared stationary (I128
    pair), removing all DVE fold trees:
      bank B: tier2 -> (read S2) -> continue tier1 -> S12
      bank A: tier0 -> S0;   bank T: tanh(msg) fp8 pairs -> T1
    s0 = S0 + S12, mdis = S2*inv2, mloc = S0*inv0, agg = T1*inv1.
  * msg matmuls: bf16 kron4(W_msg) stationary x fp8 slots (mixed dtypes),
    two slots into a 2-bank PSUM tile, ONE wide ACT tanh per pair writing
    fp8 pairs consumed by the DR tanh-sum chain.
  * All biases are zero by spec -> dropped (asserted at staging); the
    sigmoid 1/2 scale is folded into W_upd so local/upd share plain tanh.
  * Gates are normalized (exp * 1/sum) on their 12-row form, then
    broadcast to all 128 partitions by SBUF->SBUF DMA with a 0-stride
    partition AP (no scatter matmuls; combine multiplies run at DVE 2x).
  * Scheduling: per iteration the DR chains + msg pairs + tanh-sum mms
    form a PE "fill" stream; the previous tile's expert/gating/CNF units
    are woven into it in pairs.  Chain-freeing DVE reads sit right after
    each chain stop so banks recycle early; input DMA is prefetched one
    iteration ahead; bands are ordered small-at-both-ends to shorten the
    DMA-bound startup and the drain tail.  This keeps the PE dense enough
    to limit HAM re-throttling (the 1.2 GHz cold-clock trap).

PSUM banks: chains {A,B,T} on 2 banks + msg pairs 2x[128,2SC] (4) +
experts 2 = 8 banks.
"""

import numpy as np
import ml_dtypes
from contextlib import ExitStack

import concourse.bass as bass
import concourse.bacc as bacc
import concourse.tile as tile
import concourse.mybir as mybir

B, K, D, NH = 262144, 26, 32, 32
N_CORES = 8
BS = B // N_CORES          # 32768 cells per core
ST = 2048                  # cells per super-tile
NT = BS // ST              # 16 super-tiles per core
TPS = ST // 128            # 16 tiles of 128 cells per super-tile
SC = TPS * 32              # 512 free columns per super-tile (t, c)
N_STEPS = 3
DT_STEP = 1.0 / N_STEPS

dt = mybir.dt
bf16 = ml_dtypes.bfloat16
f8e4 = ml_dtypes.float8_e4m3
AF = mybir.ActivationFunctionType
ALU = mybir.AluOpType
PM = mybir.MatmulPerfMode

# bf16 stationary slots in wc: [128, n*128 + 12 + 12 + 3*128]
_WSLOTS = ["W4msg", "Wl_t", "Wl_b", "Wu_t", "Wu_b", "Wc_t", "Wc_b",
           "Wg1_t", "Wg1_b", "Wc_td"]
EX_G2 = 128 * len(_WSLOTS)          # kron(I4, W_g2): [128, 12]
EX_ONES = EX_G2 + 12                # ones12: [12, 12] group-sum bcast
EX_SCAT = EX_ONES + 12              # gate scatter e=0..2: [12, 128] each
WC_COLS = EX_SCAT + 3 * 128


def _wslot(name):
    return 128 * _WSLOTS.index(name)


def build_program(widths):
    nc = bacc.Bacc("TRN2", target_bir_lowering=False, debug=False,
                   num_devices=N_CORES)

    totc = sum(v0 + v1 + v2 for v0, v1, v2 in zip(*widths))
    a_ma = nc.dram_tensor("ma", [128, totc * SC], dt.float8e4,
                          kind="ExternalInput").ap()
    a_ci = nc.dram_tensor("ci", [128, NT * 4 * SC], dt.bfloat16,
                          kind="ExternalInput").ap()
    a_wc = nc.dram_tensor("wc", [128, WC_COLS], dt.bfloat16,
                          kind="ExternalInput").ap()
    a_wdr = nc.dram_tensor("wdr", [128, 2 * 128], dt.float8e4,
                           kind="ExternalInput").ap()
    a_out = nc.dram_tensor("out", [128, NT * SC], dt.bfloat16,
                           kind="ExternalOutput").ap()

    with tile.TileContext(nc) as tc:
        _body(tc, a_ma, a_ci, a_wc, a_wdr, a_out, widths)
    nc.compile()
    return nc


def _body(tc, a_ma, a_ci, a_wc, a_wdr, a_out, widths):
    nc = tc.nc
    w0s, w1s, w2s = widths

    with ExitStack() as ctx:
        cpool = ctx.enter_context(tc.tile_pool(name="const", bufs=1))
        pma = ctx.enter_context(tc.tile_pool(name="ma", bufs=2))
        pci = ctx.enter_context(tc.tile_pool(name="ci", bufs=2))
        ptnh = ctx.enter_context(tc.tile_pool(name="tnh", bufs=2))
        psml = ctx.enter_context(tc.tile_pool(name="sml", bufs=3))
        pout = ctx.enter_context(tc.tile_pool(name="out", bufs=2))
        # PSUM: chain pool {A, B, T1} over 2 banks (T1 reuses A's bank after
        # mloc/s0 free it) + msg pairs 2x2 + experts 2 = 8 banks.  The expert
        # pool is double-buffered so consecutive expert units overlap instead
        # of serializing on one bank (that serial chain was the critical path).
        ppCH = ctx.enter_context(tc.tile_pool(name="pCH", bufs=2, space="PSUM"))
        ppM = ctx.enter_context(tc.tile_pool(name="pM", bufs=2, space="PSUM"))
        ppE = ctx.enter_context(tc.tile_pool(name="pE", bufs=2, space="PSUM"))

        wc = cpool.tile([128, WC_COLS], dt.bfloat16, tag="wc")
        nc.sync.dma_start(wc[:], a_wc)
        wdr = cpool.tile([128, 2, 128], dt.float8e4, tag="wdr")
        nc.sync.dma_start(wdr[:], a_wdr)

        def W(name):
            return wc[:, _wslot(name): _wslot(name) + 128]

        kron_g2 = wc[:, EX_G2:EX_G2 + 12]
        ones12 = wc[0:12, EX_ONES:EX_ONES + 12]
        scat = [wc[0:12, EX_SCAT + 128 * e: EX_SCAT + 128 * (e + 1)]
                for e in range(3)]

        off = [0]

        def emit_loads(i):
            v0, v1, v2 = w0s[i], w1s[i], w2s[i]
            vt = v0 + v1 + v2
            ma = pma.tile([128, vt, SC], dt.float8e4, tag="ma")
            if i == 0:
                o01 = v0 + v1
                nc.sync.dma_start(
                    ma[:, o01:vt, :],
                    a_ma[:, (off[0] + o01) * SC:(off[0] + vt) * SC])
                nc.sync.dma_start(
                    ma[:, 0:o01, :],
                    a_ma[:, off[0] * SC:(off[0] + o01) * SC])
            else:
                nc.sync.dma_start(ma[:], a_ma[:, off[0] * SC:(off[0] + vt) * SC])
            off[0] += vt
            ci = pci.tile([128, 4, SC], dt.bfloat16, tag="ci")
            nc.sync.dma_start(ci[:], a_ci[:, i * 4 * SC:(i + 1) * 4 * SC])
            tnh = ptnh.tile([128, max(w1s), SC], dt.float8e4, tag="tnh")
            return dict(ma=ma, ci=ci, tnh=tnh, v0=v0, v1=v1, v2=v2,
                        cst=ci[:, 0, :], inv0=ci[:, 1, :], inv1=ci[:, 2, :],
                        inv2=ci[:, 3, :])

        def alloc_chain_banks(st):
            st["pA"] = ppCH.tile([128, SC], dt.float32, name="pA", tag="ch")
            st["pB"] = ppCH.tile([128, SC], dt.float32, name="pB", tag="ch")

        def chain_steps(st):
            """DR chain thunks B(tier2) | A(tier0) | B(tier1 cont) with the
            PSUM-freeing DVE reads placed right after each stop."""
            ma, v0, v1, v2 = st["ma"], st["v0"], st["v1"], st["v2"]
            pA, pB = st["pA"], st["pB"]

            def mmB1(p):
                o = v0 + v1
                return lambda: nc.tensor.matmul(
                    pB[:], wdr[:], ma[:, o + 2 * p:o + 2 * p + 2, :],
                    start=(p == 0), stop=(p == v2 // 2 - 1),
                    perf_mode=PM.DoubleRow)

            def mmA(p):
                return lambda: nc.tensor.matmul(
                    pA[:], wdr[:], ma[:, 2 * p:2 * p + 2, :],
                    start=(p == 0), stop=(p == v0 // 2 - 1),
                    perf_mode=PM.DoubleRow)

            def mmB2(p):
                return lambda: nc.tensor.matmul(
                    pB[:], wdr[:], ma[:, v0 + 2 * p:v0 + 2 * p + 2, :],
                    start=False, stop=(p == v1 // 2 - 1),
                    perf_mode=PM.DoubleRow, skip_group_check=True)

            def rd_s2():
                s2c = psml.tile([128, SC], dt.bfloat16, tag="s2c")
                nc.vector.tensor_copy(s2c[:], pB[:])
                mdis = psml.tile([128, SC], dt.bfloat16, tag="mdis")
                nc.vector.tensor_tensor(out=mdis[:], in0=s2c[:], in1=st["inv2"],
                                        op=ALU.mult)
                st["mdis"] = mdis

            def rd_A():
                mloc = psml.tile([128, SC], dt.bfloat16, tag="mloc")
                nc.vector.tensor_tensor(out=mloc[:], in0=pA[:], in1=st["inv0"],
                                        op=ALU.mult)
                st["mloc"] = mloc

            def rd_B():
                s12c = psml.tile([128, SC], dt.bfloat16, tag="s12c")
                nc.vector.tensor_copy(s12c[:], pB[:])
                s0 = psml.tile([128, SC], dt.bfloat16, tag="s0")
                nc.vector.tensor_tensor(out=s0[:], in0=pA[:], in1=s12c[:],
                                        op=ALU.add)
                st["s0"] = s0

            return ([mmB1(p) for p in range(v2 // 2)] + [rd_s2]
                    + [mmA(p) for p in range(v0 // 2)] + [rd_A]
                    + [mmB2(p) for p in range(v1 // 2)] + [rd_B])

        def tnh_mm(st, p):
            """One DR matmul accumulating tanh pair p into bank T (reuses
            A's bank slot); the last one also emits the aggb read."""
            tnh, v1 = st["tnh"], st["v1"]

            def f():
                if p == 0:
                    st["pC"] = ppCH.tile([128, SC], dt.float32, name="pC",
                                         tag="ch")
                pC = st["pC"]
                nc.tensor.matmul(pC[:], wdr[:], tnh[:, 2 * p:2 * p + 2, :],
                                 start=(p == 0), stop=(p == v1 // 2 - 1),
                                 perf_mode=PM.DoubleRow)
                if p == v1 // 2 - 1:
                    aggb = psml.tile([128, SC], dt.bfloat16, tag="aggb")
                    nc.vector.tensor_tensor(out=aggb[:], in0=pC[:],
                                            in1=st["inv1"], op=ALU.mult)
                    st["aggb"] = aggb
            return f

        def emit_msg_pair(st, p):
            """Two msg matmuls (bf16 W x fp8 slot) + one wide tanh -> fp8."""
            ma, tnh, v0 = st["ma"], st["tnh"], st["v0"]
            pm = ppM.tile([128, 2, SC], dt.float32, tag="pm")
            j = v0 + 2 * p
            nc.tensor.matmul(pm[:, 0, :], W("W4msg"), ma[:, j:j + 1, :],
                             start=True, stop=True)
            nc.tensor.matmul(pm[:, 1, :], W("W4msg"), ma[:, j + 1:j + 2, :],
                             start=True, stop=True)
            nc.scalar.activation(tnh[:, 2 * p:2 * p + 2, :], pm[:], AF.Tanh)

        def back_units(i, h):
            """Experts/gating/cnf/combine for super-tile i (chain reads done)."""
            cst = h["cst"]
            units = []

            def u_local():
                pl = ppE.tile([128, SC], dt.float32, tag="pe")
                nc.tensor.matmul(pl[:], W("Wl_t"), cst, start=True, stop=False)
                nc.tensor.matmul(pl[:], W("Wl_b"), h["mloc"][:], start=False, stop=True)
                h["locb"] = psml.tile([128, SC], dt.bfloat16, name="locb", tag="locb")
                nc.scalar.activation(h["locb"][:], pl[:], AF.Tanh)
            units.append(u_local)

            def u_func1():
                pu = ppE.tile([128, SC], dt.float32, tag="pe")
                nc.tensor.matmul(pu[:], W("Wu_t"), cst, start=True, stop=False)
                nc.tensor.matmul(pu[:], W("Wu_b"), h["aggb"][:], start=False, stop=True)
                h["tu"] = psml.tile([128, SC], dt.bfloat16, name="tu", tag="tu")
                nc.scalar.activation(h["tu"][:], pu[:], AF.Tanh)
                h["tagg"] = psml.tile([128, SC], dt.bfloat16, name="tagg", tag="tagg")
                nc.scalar.activation(h["tagg"][:], h["aggb"][:], AF.Tanh)
            units.append(u_func1)

            def u_func2():
                d2 = psml.tile([128, SC], dt.bfloat16, tag="d2")
                nc.vector.tensor_tensor(out=d2[:], in0=h["tagg"][:], in1=cst,
                                        op=ALU.subtract)
                e1 = psml.tile([128, SC], dt.bfloat16, tag="e1")
                nc.vector.tensor_tensor(out=e1[:], in0=h["tu"][:], in1=d2[:],
                                        op=ALU.mult)
                e2 = psml.tile([128, SC], dt.bfloat16, tag="e2")
                nc.vector.tensor_tensor(out=e2[:], in0=e1[:], in1=d2[:],
                                        op=ALU.add)
                h["funcb"] = psml.tile([128, SC], dt.bfloat16, name="funcb", tag="funcb")
                nc.vector.scalar_tensor_tensor(out=h["funcb"][:], in0=e2[:],
                                               scalar=0.5, in1=cst,
                                               op0=ALU.mult, op1=ALU.add)
            units.append(u_func2)

            def u_gate1():
                pg = ppE.tile([128, SC], dt.float32, tag="pe")
                nc.tensor.matmul(pg[:], W("Wg1_t"), cst, start=True, stop=False)
                nc.tensor.matmul(pg[:], W("Wg1_b"), h["s0"][:], start=False, stop=True)
                h["hb"] = psml.tile([128, SC], dt.bfloat16, name="hb", tag="hb")
                nc.vector.tensor_scalar(out=h["hb"][:], in0=pg[:], scalar1=0.0,
                                        scalar2=None, op0=ALU.max)
            units.append(u_gate1)

            def u_cnf0():
                pcnf = ppE.tile([128, SC], dt.float32, tag="pe")
                nc.tensor.matmul(pcnf[:], W("Wc_t"), cst, start=True, stop=False)
                nc.tensor.matmul(pcnf[:], W("Wc_b"), h["mdis"][:], start=False, stop=True)
                h["pcnf"] = pcnf
                h["t0"] = psml.tile([128, SC], dt.bfloat16, name="t0", tag="t0")
                nc.scalar.activation(h["t0"][:], pcnf[:], AF.Tanh)
                nc.tensor.matmul(pcnf[:], W("Wc_td"), h["t0"][:], start=False,
                                 stop=True, skip_group_check=True)
            units.append(u_cnf0)

            def u_cnf1():
                pcnf = h["pcnf"]
                h["t1"] = psml.tile([128, SC], dt.bfloat16, name="t1", tag="t1")
                nc.scalar.activation(h["t1"][:], pcnf[:], AF.Tanh)
                nc.tensor.matmul(pcnf[:], W("Wc_td"), h["t1"][:], start=False,
                                 stop=True, skip_group_check=True)
                h["t01"] = psml.tile([128, SC], dt.bfloat16, name="t01", tag="t01")
                nc.gpsimd.tensor_tensor(out=h["t01"][:], in0=h["t0"][:],
                                        in1=h["t1"][:], op=ALU.add)
            units.append(u_cnf1)

            def u_cnf2():
                pcnf = h["pcnf"]
                t2 = psml.tile([128, SC], dt.bfloat16, tag="t2")
                nc.scalar.activation(t2[:], pcnf[:], AF.Tanh)
                tall = psml.tile([128, SC], dt.bfloat16, tag="tall")
                nc.gpsimd.tensor_tensor(out=tall[:], in0=h["t01"][:], in1=t2[:],
                                        op=ALU.add)
                h["dist"] = psml.tile([128, SC], dt.bfloat16, name="dist", tag="dist")
                nc.vector.scalar_tensor_tensor(out=h["dist"][:], in0=tall[:],
                                               scalar=DT_STEP, in1=cst,
                                               op0=ALU.mult, op1=ALU.add)
            units.append(u_cnf2)

            def u_gate2():
                pl2 = ppE.tile([128, SC], dt.float32, tag="pe")
                nc.tensor.matmul(pl2[0:12, :], kron_g2, h["hb"][:], start=True,
                                 stop=True)
                h["eg"] = psml.tile([12, SC], dt.bfloat16, name="eg", tag="eg")
                nc.scalar.activation(h["eg"][:], pl2[0:12, :], AF.Exp)
                ps = ppE.tile([128, SC], dt.float32, tag="pe")
                nc.tensor.matmul(ps[0:12, :], ones12, h["eg"][:], start=True,
                                 stop=True)
                rec = psml.tile([12, SC], dt.float32, tag="rec")
                nc.vector.reciprocal_approx_fast(out=rec[:], in_=ps[0:12, :])
                h["egn"] = psml.tile([12, SC], dt.bfloat16, name="egn", tag="egn")
                nc.vector.tensor_tensor(out=h["egn"][:], in0=h["eg"][:],
                                        in1=rec[:], op=ALU.mult)
            units.append(u_gate2)

            def u_comb():
                exps = [h["locb"], h["funcb"], h["dist"]]
                aes = []
                for e in range(3):
                    # broadcast gate row e to all (g, d) partitions via DMA
                    geb = psml.tile([128, SC], dt.bfloat16, name=f"geb{e}",
                                    tag=f"geb{e}")
                    bsrc = h["egn"][e:12:3, :].unsqueeze(1).to_broadcast(
                        [4, 32, SC])
                    nc.sync.dma_start(geb[:], bsrc)
                    ae = psml.tile([128, SC], dt.bfloat16, tag=f"ae{e}")
                    nc.vector.tensor_tensor(out=ae[:], in0=geb[:],
                                            in1=exps[e][:], op=ALU.mult)
                    aes.append(ae)
                a12 = psml.tile([128, SC], dt.bfloat16, tag="a12")
                nc.gpsimd.tensor_tensor(out=a12[:], in0=aes[0][:], in1=aes[1][:],
                                        op=ALU.add)
                outb = pout.tile([128, SC], dt.bfloat16, tag="outb")
                nc.vector.tensor_tensor(out=outb[:], in0=a12[:], in1=aes[2][:],
                                        op=ALU.add)
                nc.sync.dma_start(a_out[:, i * SC:(i + 1) * SC], outb[:])
            units.append(u_comb)

            return units

        # --- main pipeline ---
        # Front phase: DR chain matmuls with msg pairs as spacers (different
        # PSUM banks, so the PE never waits on ACT evacuation).  Back phase:
        # experts(i-1) woven with the tanh-sum DR matmuls and leftover pairs.
        prev = None
        nxt = emit_loads(0)
        for i in range(NT):
            st = nxt
            if i + 1 < NT:
                nxt = emit_loads(i + 1)         # prefetch next ST's DMA
            alloc_chain_banks(st)
            npairs = st["v1"] // 2
            pairs = [lambda p=p: emit_msg_pair(st, p) for p in range(npairs)]
            if i == NT - 1:
                fill = pairs + chain_steps(st)
            else:
                fill = chain_steps(st) + pairs
            fill += [tnh_mm(st, p) for p in range(npairs)]
            units = back_units(i - 1, prev) if prev is not None else []
            # interleave units in PAIRS to halve PE weight swaps between the
            # shared chain stationary and the per-unit expert weights
            upairs = [units[j:j + 2] for j in range(0, len(units), 2)]
            n_u, n_f = len(upairs), len(fill)
            k = 0
            for n, fn in enumerate(fill):
                fn()
                while k < n_u and (k + 1) * n_f <= (n + 1) * n_u:
                    for u in upairs[k]:
                        u()
                    k += 1
            while k < n_u:
                for u in upairs[k]:
                    u()
                k += 1
            prev = st
        for u in back_units(NT - 1, prev):
            u()


# ---------------------------------------------------------------------------
# host staging
# ---------------------------------------------------------------------------

def _to_blockT(arr_bsd):
    """[n, d] (d == 32, n multiple of 128) -> blockT [128, n//128*32]."""
    n, d = arr_bsd.shape
    a = arr_bsd.reshape(n // 128, 4, 32, d)          # [t, g, c, d]
    a = a.transpose(1, 3, 0, 2)                      # [g, d, t, c]
    return np.ascontiguousarray(a.reshape(128, n // 4))


def _nb_blockT(nb_sel):
    """[n, w, 32] premasked sorted neighbors -> [128, w*n//4] (j outermost)."""
    n, w, d = nb_sel.shape
    a = nb_sel.reshape(n // 128, 4, 32, w, d)        # [t, g, c, j, d]
    a = a.transpose(1, 4, 3, 0, 2)                   # [g, d, j, t, c]
    return np.ascontiguousarray(a.reshape(128, w * n // 4))


def _from_blockT(arr):
    """inverse of _to_blockT per ST block: [128, NT*SC] -> [BS, 32]."""
    a = arr.reshape(4, 32, NT * TPS, 32)             # [g, d, t, c]
    a = a.transpose(2, 0, 3, 1)                      # [t, g, c, d]
    return np.ascontiguousarray(a.reshape(BS, 32))


def _fp8_error_feedback(x):
    """Quantize [n, w, d] to fp8e4 with per-cell carry along slot axis so
    the slot-sum is preserved to ~1 quantization step."""
    n, w, d = x.shape
    q = np.empty((n, w, d), f8e4)
    carry = np.zeros((n, d), np.float32)
    for j in range(w):
        t = x[:, j, :] + carry
        qj = t.astype(f8e4)
        q[:, j, :] = qj
        carry = t - qj.astype(np.float32)
    return q


def stage_weights(inputs):
    f32 = np.float32
    for b in ("b_local", "b_msg", "b_upd", "b_cnf", "b_g1", "b_g2"):
        if np.any(np.asarray(inputs[b], f32) != 0.0):
            raise NotImplementedError(f"kernel requires {b} == 0")

    W_local = np.asarray(inputs["W_local"], f32)
    W_msg = np.asarray(inputs["W_msg"], f32)
    W_upd = np.asarray(inputs["W_upd"], f32)
    W_cnf = np.asarray(inputs["W_cnf"], f32)
    W_g1 = np.asarray(inputs["W_g1"], f32)
    W_g2 = np.asarray(inputs["W_g2"], f32)

    eye4 = np.eye(4, dtype=f32)

    def kron4(w):
        return np.kron(eye4, w)

    wparts = {
        "W4msg": kron4(W_msg),
        "Wl_t": kron4(W_local[:D]), "Wl_b": kron4(W_local[D:]),
        "Wu_t": kron4(0.5 * W_upd[:D]), "Wu_b": kron4(0.5 * W_upd[D:]),
        "Wc_t": kron4(W_cnf[:D]), "Wc_b": kron4(W_cnf[D:]),
        "Wg1_t": kron4(W_g1[:D]), "Wg1_b": kron4(W_g1[D:] / K),
        "Wc_td": kron4(W_cnf[:D] * DT_STEP),
    }
    wc = np.zeros((128, WC_COLS), f32)
    for name in _WSLOTS:
        wc[:, _wslot(name):_wslot(name) + 128] = wparts[name]
    for g in range(4):
        wc[32 * g:32 * (g + 1), EX_G2 + 3 * g:EX_G2 + 3 * (g + 1)] = W_g2
    # ones12: [12, 12] contraction rows (g,e') -> out (g,e): 1 if same g
    for g in range(4):
        for e1 in range(3):
            for e2 in range(3):
                wc[3 * g + e1, EX_ONES + 3 * g + e2] = 1.0
    for e in range(3):
        for g in range(4):
            wc[3 * g + e, EX_SCAT + 128 * e + 32 * g:
               EX_SCAT + 128 * e + 32 * (g + 1)] = 1.0
    wc = wc.astype(bf16)

    eye = np.eye(128, dtype=f32)
    wdr = np.stack([eye, eye], axis=1).astype(f8e4)  # [128, 2, 128]
    return wc, wdr.reshape(128, 2 * 128)


def stage_inputs(inputs):
    """Returns (in_maps, widths, perm)."""
    f32 = np.float32
    cs = np.asarray(inputs["current_state"], f32)
    nb = np.asarray(inputs["neighbor_states"], f32)
    tiers = np.asarray(inputs["tier_ids"], np.int32)

    cnt = np.stack([(tiers == t).sum(-1) for t in range(3)], axis=1)  # [B,3]
    snake_c0 = np.where(cnt[:, 1] % 2 == 0, cnt[:, 0], -cnt[:, 0])
    perm = np.lexsort((snake_c0, cnt[:, 1]))         # rank -> cell

    # per-ST band widths (shared by all cores), padded to even, min 2
    wraw = []
    for t in range(3):
        ct = cnt[perm, t].reshape(NT, ST * N_CORES)
        w = np.maximum(2, ct.max(axis=1))
        wraw.append(w + (w % 2))
    # smallest bands at both ends (fast DMA-bound startup, short tail),
    # largest in the middle
    asc = np.argsort(wraw[0] + wraw[1] + wraw[2], kind="stable")
    border = np.concatenate([asc[0::2], asc[1::2][::-1]])
    widths = tuple(tuple(int(x) for x in w[border]) for w in wraw)

    inv = (1.0 / np.maximum(cnt, 1.0)).astype(f32)   # [B, 3]
    wc, wdr = stage_weights(inputs)

    wmax = [max(w) for w in widths]
    orders = []
    for t in range(3):
        order = np.argsort(tiers != t, axis=1, kind="stable")[:, :wmax[t]]
        orders.append(order)

    in_maps = []
    for c in range(N_CORES):
        cells = perm[c::N_CORES]                     # [BS] cell ids
        cello = np.concatenate([cells[b * ST:(b + 1) * ST] for b in border])
        mall = []
        for i in range(NT):
            cell_i = cello[i * ST:(i + 1) * ST]
            for t in range(3):
                w = widths[t][i]
                od = orders[t][cell_i, :w]
                sel = np.take_along_axis(nb[cell_i], od[:, :, None], axis=1)
                msk = np.take_along_axis(tiers[cell_i], od, axis=1) == t
                x = (sel * msk[:, :, None]).astype(f32)
                if t == 1:
                    q = x.astype(f8e4)               # per-slot accuracy
                else:
                    q = _fp8_error_feedback(x)       # sum accuracy
                # blockT with j outermost, fp8 via f32 view for _nb_blockT
                mall.append(_nb_blockT(q.astype(f32)).astype(f8e4))
        ci = np.empty((128, NT * 4 * SC), f32)
        for i in range(NT):
            cell_i = cello[i * ST:(i + 1) * ST]
            ci[:, (4 * i) * SC:(4 * i + 1) * SC] = _to_blockT(cs[cell_i])
            for t in range(3):
                ci[:, (4 * i + 1 + t) * SC:(4 * i + 2 + t) * SC] = \
                    _to_blockT(np.repeat(inv[cell_i, t:t + 1], D, axis=1))
        in_maps.append({
            "ma": np.concatenate(mall, axis=1),
            "ci": ci.astype(bf16),
            "wc": wc, "wdr": wdr,
        })
    return in_maps, widths, perm, border


_PROGRAM_CACHE = {}


def kernel(**inputs):
    from concourse.bass_utils import run_bass_kernel_spmd

    in_maps, widths, perm, border = stage_inputs(inputs)
    if widths not in _PROGRAM_CACHE:
        _PROGRAM_CACHE[widths] = build_program(widths)
    nc = _PROGRAM_CACHE[widths]

    res = run_bass_kernel_spmd(nc, in_maps, core_ids=list(range(N_CORES)))
    out = np.empty((B, D), np.float32)
    for c in range(N_CORES):
        cells = perm[c::N_CORES]
        cello = np.concatenate([cells[b * ST:(b + 1) * ST] for b in border])
        out[cello] = _from_blockT(
            np.asarray(res.results[c]["out"], np.float32))
    return out


# revision 34
# speedup vs baseline: 1.0316x; 1.0316x over previous
"""Trainium2 Bass kernel for nn_MoEConnectionProcessor (v3: fp8 DoubleRow chains).

Strategy (delta over v2)
------------------------
Data-parallel over 8 cores; per core 16 super-tiles (ST) of 2048 cells in
blockT layout: SBUF partition = (g, d) (cell-subgroup x feature), free
axis = (t, c) = 512 cols per ST.

v3 structural changes (415us -> ~267us measured on HW):
  * The three premasked neighbor copies are staged as ONE concatenated
    fp8e4m3 tensor [tier0 | tier1 | tier2] with per-band widths padded to
    EVEN.  m0/m2 use host-side error-feedback quantization (per-cell carry
    across slots) so the tier SUMS keep near-bf16 accuracy.
  * All slot sums run on the PE as DoubleRow fp8 accumulation chains
    (2 slots per matmul, 2x rate) with a single shared stationary (I128
    pair), removing all DVE fold trees:
      bank B: tier2 -> (read S2) -> continue tier1 -> S12
      bank A: tier0 -> S0;   bank T: tanh(msg) fp8 pairs -> T1
    s0 = S0 + S12, mdis = S2*inv2, mloc = S0*inv0, agg = T1*inv1.
  * msg matmuls: bf16 kron4(W_msg) stationary x fp8 slots (mixed dtypes),
    two slots into a 2-bank PSUM tile, ONE wide ACT tanh per pair writing
    fp8 pairs consumed by the DR tanh-sum chain.
  * All biases are zero by spec -> dropped (asserted at staging); the
    sigmoid 1/2 scale is folded into W_upd so local/upd share plain tanh.
  * Gates are normalized (exp * 1/sum) on their 12-row form, then
    broadcast to all 128 partitions by SBUF->SBUF DMA with a 0-stride
    partition AP (no scatter matmuls; combine multiplies run at DVE 2x).
  * Scheduling: per iteration the DR chains + msg pairs + tanh-sum mms
    form a PE "fill" stream; the previous tile's expert/gating/CNF units
    are woven into it in pairs.  Chain-freeing DVE reads sit right after
    each chain stop so banks recycle early; input DMA is prefetched one
    iteration ahead; bands are ordered small-at-both-ends to shorten the
    DMA-bound startup and the drain tail.  This keeps the PE dense enough
    to limit HAM re-throttling (the 1.2 GHz cold-clock trap).

PSUM banks: chains {A,B,T} on 2 banks + msg pairs 2x[128,2SC] (4) +
experts 2 = 8 banks.
"""

import numpy as np
import ml_dtypes
from contextlib import ExitStack

import concourse.bass as bass
import concourse.bacc as bacc
import concourse.tile as tile
import concourse.mybir as mybir

B, K, D, NH = 262144, 26, 32, 32
N_CORES = 8
BS = B // N_CORES          # 32768 cells per core
ST = 2048                  # cells per super-tile
NT = BS // ST              # 16 super-tiles per core
TPS = ST // 128            # 16 tiles of 128 cells per super-tile
SC = TPS * 32              # 512 free columns per super-tile (t, c)
N_STEPS = 3
DT_STEP = 1.0 / N_STEPS

dt = mybir.dt
bf16 = ml_dtypes.bfloat16
f8e4 = ml_dtypes.float8_e4m3
AF = mybir.ActivationFunctionType
ALU = mybir.AluOpType
PM = mybir.MatmulPerfMode

# bf16 stationary slots in wc: [128, n*128 + 12 + 12 + 3*128]
_WSLOTS = ["W4msg", "Wl_t", "Wl_b", "Wu_t", "Wu_b", "Wc_t", "Wc_b",
           "Wg1_t", "Wg1_b", "Wc_td"]
EX_G2 = 128 * len(_WSLOTS)          # kron(I4, W_g2): [128, 12]
EX_ONES = EX_G2 + 12                # ones12: [12, 12] group-sum bcast
EX_SCAT = EX_ONES + 12              # gate scatter e=0..2: [12, 128] each
WC_COLS = EX_SCAT + 3 * 128


def _wslot(name):
    return 128 * _WSLOTS.index(name)


def build_program(widths):
    nc = bacc.Bacc("TRN2", target_bir_lowering=False, debug=False,
                   num_devices=N_CORES)

    totc = sum(v0 + v1 + v2 for v0, v1, v2 in zip(*widths))
    a_ma = nc.dram_tensor("ma", [128, totc * SC], dt.float8e4,
                          kind="ExternalInput").ap()
    a_ci = nc.dram_tensor("ci", [128, NT * 4 * SC], dt.bfloat16,
                          kind="ExternalInput").ap()
    a_wc = nc.dram_tensor("wc", [128, WC_COLS], dt.bfloat16,
                          kind="ExternalInput").ap()
    a_wdr = nc.dram_tensor("wdr", [128, 2 * 128], dt.float8e4,
                           kind="ExternalInput").ap()
    a_out = nc.dram_tensor("out", [128, NT * SC], dt.bfloat16,
                           kind="ExternalOutput").ap()

    with tile.TileContext(nc) as tc:
        _body(tc, a_ma, a_ci, a_wc, a_wdr, a_out, widths)
    nc.compile()
    return nc


def _body(tc, a_ma, a_ci, a_wc, a_wdr, a_out, widths):
    nc = tc.nc
    w0s, w1s, w2s = widths

    with ExitStack() as ctx:
        cpool = ctx.enter_context(tc.tile_pool(name="const", bufs=1))
        pma = ctx.enter_context(tc.tile_pool(name="ma", bufs=2))
        pci = ctx.enter_context(tc.tile_pool(name="ci", bufs=2))
        ptnh = ctx.enter_context(tc.tile_pool(name="tnh", bufs=2))
        psml = ctx.enter_context(tc.tile_pool(name="sml", bufs=3))
        pout = ctx.enter_context(tc.tile_pool(name="out", bufs=2))
        # PSUM: chain pool {A, B, T1} over 2 banks (T1 reuses A's bank after
        # mloc/s0 free it) + msg pairs 2x2 + experts 2 = 8 banks.  The expert
        # pool is double-buffered so consecutive expert units overlap instead
        # of serializing on one bank (that serial chain was the critical path).
        ppCH = ctx.enter_context(tc.tile_pool(name="pCH", bufs=2, space="PSUM"))
        ppM = ctx.enter_context(tc.tile_pool(name="pM", bufs=2, space="PSUM"))
        ppE = ctx.enter_context(tc.tile_pool(name="pE", bufs=2, space="PSUM"))

        wc = cpool.tile([128, WC_COLS], dt.bfloat16, tag="wc")
        nc.sync.dma_start(wc[:], a_wc)
        wdr = cpool.tile([128, 2, 128], dt.float8e4, tag="wdr")
        nc.sync.dma_start(wdr[:], a_wdr)

        def W(name):
            return wc[:, _wslot(name): _wslot(name) + 128]

        kron_g2 = wc[:, EX_G2:EX_G2 + 12]
        ones12 = wc[0:12, EX_ONES:EX_ONES + 12]
        scat = [wc[0:12, EX_SCAT + 128 * e: EX_SCAT + 128 * (e + 1)]
                for e in range(3)]

        off = [0]

        def emit_loads(i):
            v0, v1, v2 = w0s[i], w1s[i], w2s[i]
            vt = v0 + v1 + v2
            ma = pma.tile([128, vt, SC], dt.float8e4, tag="ma")
            if i == 0:
                o01 = v0 + v1
                nc.sync.dma_start(
                    ma[:, o01:vt, :],
                    a_ma[:, (off[0] + o01) * SC:(off[0] + vt) * SC])
                nc.sync.dma_start(
                    ma[:, 0:o01, :],
                    a_ma[:, off[0] * SC:(off[0] + o01) * SC])
            else:
                nc.sync.dma_start(ma[:], a_ma[:, off[0] * SC:(off[0] + vt) * SC])
            off[0] += vt
            ci = pci.tile([128, 4, SC], dt.bfloat16, tag="ci")
            nc.sync.dma_start(ci[:], a_ci[:, i * 4 * SC:(i + 1) * 4 * SC])
            tnh = ptnh.tile([128, max(w1s), SC], dt.float8e4, tag="tnh")
            return dict(ma=ma, ci=ci, tnh=tnh, v0=v0, v1=v1, v2=v2,
                        cst=ci[:, 0, :], inv0=ci[:, 1, :], inv1=ci[:, 2, :],
                        inv2=ci[:, 3, :])

        def alloc_chain_banks(st):
            st["pA"] = ppCH.tile([128, SC], dt.float32, name="pA", tag="ch")
            st["pB"] = ppCH.tile([128, SC], dt.float32, name="pB", tag="ch")

        def chain_steps(st):
            """DR chain thunks B(tier2) | A(tier0) | B(tier1 cont) with the
            PSUM-freeing DVE reads placed right after each stop."""
            ma, v0, v1, v2 = st["ma"], st["v0"], st["v1"], st["v2"]
            pA, pB = st["pA"], st["pB"]

            def mmB1(p):
                o = v0 + v1
                return lambda: nc.tensor.matmul(
                    pB[:], wdr[:], ma[:, o + 2 * p:o + 2 * p + 2, :],
                    start=(p == 0), stop=(p == v2 // 2 - 1),
                    perf_mode=PM.DoubleRow)

            def mmA(p):
                return lambda: nc.tensor.matmul(
                    pA[:], wdr[:], ma[:, 2 * p:2 * p + 2, :],
                    start=(p == 0), stop=(p == v0 // 2 - 1),
                    perf_mode=PM.DoubleRow)

            def mmB2(p):
                return lambda: nc.tensor.matmul(
                    pB[:], wdr[:], ma[:, v0 + 2 * p:v0 + 2 * p + 2, :],
                    start=False, stop=(p == v1 // 2 - 1),
                    perf_mode=PM.DoubleRow, skip_group_check=True)

            def rd_s2():
                s2c = psml.tile([128, SC], dt.bfloat16, tag="s2c")
                nc.vector.tensor_copy(s2c[:], pB[:])
                mdis = psml.tile([128, SC], dt.bfloat16, tag="mdis")
                nc.vector.tensor_tensor(out=mdis[:], in0=s2c[:], in1=st["inv2"],
                                        op=ALU.mult)
                st["mdis"] = mdis

            def rd_A():
                mloc = psml.tile([128, SC], dt.bfloat16, tag="mloc")
                nc.vector.tensor_tensor(out=mloc[:], in0=pA[:], in1=st["inv0"],
                                        op=ALU.mult)
                st["mloc"] = mloc

            def rd_B():
                s12c = psml.tile([128, SC], dt.bfloat16, tag="s12c")
                nc.vector.tensor_copy(s12c[:], pB[:])
                s0 = psml.tile([128, SC], dt.bfloat16, tag="s0")
                nc.vector.tensor_tensor(out=s0[:], in0=pA[:], in1=s12c[:],
                                        op=ALU.add)
                st["s0"] = s0

            return ([mmB1(p) for p in range(v2 // 2)] + [rd_s2]
                    + [mmA(p) for p in range(v0 // 2)] + [rd_A]
                    + [mmB2(p) for p in range(v1 // 2)] + [rd_B])

        def tnh_mm(st, p):
            """One DR matmul accumulating tanh pair p into bank T (reuses
            A's bank slot); the last one also emits the aggb read."""
            tnh, v1 = st["tnh"], st["v1"]

            def f():
                if p == 0:
                    st["pC"] = ppCH.tile([128, SC], dt.float32, name="pC",
                                         tag="ch")
                pC = st["pC"]
                nc.tensor.matmul(pC[:], wdr[:], tnh[:, 2 * p:2 * p + 2, :],
                                 start=(p == 0), stop=(p == v1 // 2 - 1),
                                 perf_mode=PM.DoubleRow)
                if p == v1 // 2 - 1:
                    aggb = psml.tile([128, SC], dt.bfloat16, tag="aggb")
                    nc.vector.tensor_tensor(out=aggb[:], in0=pC[:],
                                            in1=st["inv1"], op=ALU.mult)
                    st["aggb"] = aggb
            return f

        def emit_msg_pair(st, p):
            """Two msg matmuls (bf16 W x fp8 slot) + one wide tanh -> fp8."""
            ma, tnh, v0 = st["ma"], st["tnh"], st["v0"]
            pm = ppM.tile([128, 2, SC], dt.float32, tag="pm")
            j = v0 + 2 * p
            nc.tensor.matmul(pm[:, 0, :], W("W4msg"), ma[:, j:j + 1, :],
                             start=True, stop=True)
            nc.tensor.matmul(pm[:, 1, :], W("W4msg"), ma[:, j + 1:j + 2, :],
                             start=True, stop=True)
            nc.scalar.activation(tnh[:, 2 * p:2 * p + 2, :], pm[:], AF.Tanh)

        def back_units(i, h):
            """Experts/gating/cnf/combine for super-tile i (chain reads done)."""
            cst = h["cst"]
            units = []

            def u_local():
                pl = ppE.tile([128, SC], dt.float32, tag="pe")
                nc.tensor.matmul(pl[:], W("Wl_t"), cst, start=True, stop=False)
                nc.tensor.matmul(pl[:], W("Wl_b"), h["mloc"][:], start=False, stop=True)
                h["locb"] = psml.tile([128, SC], dt.bfloat16, name="locb", tag="locb")
                nc.scalar.activation(h["locb"][:], pl[:], AF.Tanh)
            units.append(u_local)

            def u_func1():
                pu = ppE.tile([128, SC], dt.float32, tag="pe")
                nc.tensor.matmul(pu[:], W("Wu_t"), cst, start=True, stop=False)
                nc.tensor.matmul(pu[:], W("Wu_b"), h["aggb"][:], start=False, stop=True)
                h["tu"] = psml.tile([128, SC], dt.bfloat16, name="tu", tag="tu")
                nc.scalar.activation(h["tu"][:], pu[:], AF.Tanh)
                h["tagg"] = psml.tile([128, SC], dt.bfloat16, name="tagg", tag="tagg")
                nc.scalar.activation(h["tagg"][:], h["aggb"][:], AF.Tanh)
            units.append(u_func1)

            def u_func2():
                d2 = psml.tile([128, SC], dt.bfloat16, tag="d2")
                nc.vector.tensor_tensor(out=d2[:], in0=h["tagg"][:], in1=cst,
                                        op=ALU.subtract)
                e1 = psml.tile([128, SC], dt.bfloat16, tag="e1")
                nc.vector.tensor_tensor(out=e1[:], in0=h["tu"][:], in1=d2[:],
                                        op=ALU.mult)
                e2 = psml.tile([128, SC], dt.bfloat16, tag="e2")
                nc.vector.tensor_tensor(out=e2[:], in0=e1[:], in1=d2[:],
                                        op=ALU.add)
                h["funcb"] = psml.tile([128, SC], dt.bfloat16, name="funcb", tag="funcb")
                nc.vector.scalar_tensor_tensor(out=h["funcb"][:], in0=e2[:],
                                               scalar=0.5, in1=cst,
                                               op0=ALU.mult, op1=ALU.add)
            units.append(u_func2)

            def u_gate1():
                pg = ppE.tile([128, SC], dt.float32, tag="pe")
                nc.tensor.matmul(pg[:], W("Wg1_t"), cst, start=True, stop=False)
                nc.tensor.matmul(pg[:], W("Wg1_b"), h["s0"][:], start=False, stop=True)
                h["hb"] = psml.tile([128, SC], dt.bfloat16, name="hb", tag="hb")
                nc.vector.tensor_scalar(out=h["hb"][:], in0=pg[:], scalar1=0.0,
                                        scalar2=None, op0=ALU.max)
            units.append(u_gate1)

            def u_cnf0():
                pcnf = ppE.tile([128, SC], dt.float32, tag="pe")
                nc.tensor.matmul(pcnf[:], W("Wc_t"), cst, start=True, stop=False)
                nc.tensor.matmul(pcnf[:], W("Wc_b"), h["mdis"][:], start=False, stop=True)
                h["pcnf"] = pcnf
                h["t0"] = psml.tile([128, SC], dt.bfloat16, name="t0", tag="t0")
                nc.scalar.activation(h["t0"][:], pcnf[:], AF.Tanh)
                nc.tensor.matmul(pcnf[:], W("Wc_td"), h["t0"][:], start=False,
                                 stop=True, skip_group_check=True)
            units.append(u_cnf0)

            def u_cnf1():
                pcnf = h["pcnf"]
                h["t1"] = psml.tile([128, SC], dt.bfloat16, name="t1", tag="t1")
                nc.scalar.activation(h["t1"][:], pcnf[:], AF.Tanh)
                nc.tensor.matmul(pcnf[:], W("Wc_td"), h["t1"][:], start=False,
                                 stop=True, skip_group_check=True)
                h["t01"] = psml.tile([128, SC], dt.bfloat16, name="t01", tag="t01")
                nc.gpsimd.tensor_tensor(out=h["t01"][:], in0=h["t0"][:],
                                        in1=h["t1"][:], op=ALU.add)
            units.append(u_cnf1)

            def u_cnf2():
                pcnf = h["pcnf"]
                t2 = psml.tile([128, SC], dt.bfloat16, tag="t2")
                nc.scalar.activation(t2[:], pcnf[:], AF.Tanh)
                tall = psml.tile([128, SC], dt.bfloat16, tag="tall")
                nc.gpsimd.tensor_tensor(out=tall[:], in0=h["t01"][:], in1=t2[:],
                                        op=ALU.add)
                h["dist"] = psml.tile([128, SC], dt.bfloat16, name="dist", tag="dist")
                nc.vector.scalar_tensor_tensor(out=h["dist"][:], in0=tall[:],
                                               scalar=DT_STEP, in1=cst,
                                               op0=ALU.mult, op1=ALU.add)
            units.append(u_cnf2)

            def u_gate2():
                pl2 = ppE.tile([128, SC], dt.float32, tag="pe")
                nc.tensor.matmul(pl2[0:12, :], kron_g2, h["hb"][:], start=True,
                                 stop=True)
                h["eg"] = psml.tile([12, SC], dt.bfloat16, name="eg", tag="eg")
                nc.scalar.activation(h["eg"][:], pl2[0:12, :], AF.Exp)
                ps = ppE.tile([128, SC], dt.float32, tag="pe")
                nc.tensor.matmul(ps[0:12, :], ones12, h["eg"][:], start=True,
                                 stop=True)
                rec = psml.tile([12, SC], dt.float32, tag="rec")
                nc.vector.reciprocal_approx_fast(out=rec[:], in_=ps[0:12, :])
                h["egn"] = psml.tile([12, SC], dt.bfloat16, name="egn", tag="egn")
                nc.vector.tensor_tensor(out=h["egn"][:], in0=h["eg"][:],
                                        in1=rec[:], op=ALU.mult)
            units.append(u_gate2)

            def u_comb():
                exps = [h["locb"], h["funcb"], h["dist"]]
                aes = []
                for e in range(3):
                    # broadcast gate row e to all (g, d) partitions via DMA
                    geb = psml.tile([128, SC], dt.bfloat16, name=f"geb{e}",
                                    tag=f"geb{e}")
                    bsrc = h["egn"][e:12:3, :].unsqueeze(1).to_broadcast(
                        [4, 32, SC])
                    nc.sync.dma_start(geb[:], bsrc)
                    ae = psml.tile([128, SC], dt.bfloat16, tag=f"ae{e}")
                    nc.vector.tensor_tensor(out=ae[:], in0=geb[:],
                                            in1=exps[e][:], op=ALU.mult)
                    aes.append(ae)
                a12 = psml.tile([128, SC], dt.bfloat16, tag="a12")
                nc.gpsimd.tensor_tensor(out=a12[:], in0=aes[0][:], in1=aes[1][:],
                                        op=ALU.add)
                outb = pout.tile([128, SC], dt.bfloat16, tag="outb")
                nc.vector.tensor_tensor(out=outb[:], in0=a12[:], in1=aes[2][:],
                                        op=ALU.add)
                nc.sync.dma_start(a_out[:, i * SC:(i + 1) * SC], outb[:])
            units.append(u_comb)

            return units

        # --- main pipeline ---
        # Front phase: DR chain matmuls with msg pairs as spacers (different
        # PSUM banks, so the PE never waits on ACT evacuation).  Back phase:
        # experts(i-1) woven with the tanh-sum DR matmuls and leftover pairs.
        prev = None
        nxt = emit_loads(0)
        for i in range(NT):
            st = nxt
            if i + 1 < NT:
                nxt = emit_loads(i + 1)         # prefetch next ST's DMA
            alloc_chain_banks(st)
            npairs = st["v1"] // 2
            fill = chain_steps(st)
            fill += [lambda p=p: emit_msg_pair(st, p) for p in range(npairs)]
            fill += [tnh_mm(st, p) for p in range(npairs)]
            units = back_units(i - 1, prev) if prev is not None else []
            # interleave units in PAIRS to halve PE weight swaps between the
            # shared chain stationary and the per-unit expert weights
            upairs = [units[j:j + 2] for j in range(0, len(units), 2)]
            n_u, n_f = len(upairs), len(fill)
            k = 0
            for n, fn in enumerate(fill):
                fn()
                while k < n_u and (k + 1) * n_f <= (n + 1) * n_u:
                    for u in upairs[k]:
                        u()
                    k += 1
            while k < n_u:
                for u in upairs[k]:
                    u()
                k += 1
            prev = st
        for u in back_units(NT - 1, prev):
            u()


# ---------------------------------------------------------------------------
# host staging
# ---------------------------------------------------------------------------

def _to_blockT(arr_bsd):
    """[n, d] (d == 32, n multiple of 128) -> blockT [128, n//128*32]."""
    n, d = arr_bsd.shape
    a = arr_bsd.reshape(n // 128, 4, 32, d)          # [t, g, c, d]
    a = a.transpose(1, 3, 0, 2)                      # [g, d, t, c]
    return np.ascontiguousarray(a.reshape(128, n // 4))


def _nb_blockT(nb_sel):
    """[n, w, 32] premasked sorted neighbors -> [128, w*n//4] (j outermost)."""
    n, w, d = nb_sel.shape
    a = nb_sel.reshape(n // 128, 4, 32, w, d)        # [t, g, c, j, d]
    a = a.transpose(1, 4, 3, 0, 2)                   # [g, d, j, t, c]
    return np.ascontiguousarray(a.reshape(128, w * n // 4))


def _from_blockT(arr):
    """inverse of _to_blockT per ST block: [128, NT*SC] -> [BS, 32]."""
    a = arr.reshape(4, 32, NT * TPS, 32)             # [g, d, t, c]
    a = a.transpose(2, 0, 3, 1)                      # [t, g, c, d]
    return np.ascontiguousarray(a.reshape(BS, 32))


def _fp8_error_feedback(x):
    """Quantize [n, w, d] to fp8e4 with per-cell carry along slot axis so
    the slot-sum is preserved to ~1 quantization step."""
    n, w, d = x.shape
    q = np.empty((n, w, d), f8e4)
    carry = np.zeros((n, d), np.float32)
    for j in range(w):
        t = x[:, j, :] + carry
        qj = t.astype(f8e4)
        q[:, j, :] = qj
        carry = t - qj.astype(np.float32)
    return q


def stage_weights(inputs):
    f32 = np.float32
    for b in ("b_local", "b_msg", "b_upd", "b_cnf", "b_g1", "b_g2"):
        if np.any(np.asarray(inputs[b], f32) != 0.0):
            raise NotImplementedError(f"kernel requires {b} == 0")

    W_local = np.asarray(inputs["W_local"], f32)
    W_msg = np.asarray(inputs["W_msg"], f32)
    W_upd = np.asarray(inputs["W_upd"], f32)
    W_cnf = np.asarray(inputs["W_cnf"], f32)
    W_g1 = np.asarray(inputs["W_g1"], f32)
    W_g2 = np.asarray(inputs["W_g2"], f32)

    eye4 = np.eye(4, dtype=f32)

    def kron4(w):
        return np.kron(eye4, w)

    wparts = {
        "W4msg": kron4(W_msg),
        "Wl_t": kron4(W_local[:D]), "Wl_b": kron4(W_local[D:]),
        "Wu_t": kron4(0.5 * W_upd[:D]), "Wu_b": kron4(0.5 * W_upd[D:]),
        "Wc_t": kron4(W_cnf[:D]), "Wc_b": kron4(W_cnf[D:]),
        "Wg1_t": kron4(W_g1[:D]), "Wg1_b": kron4(W_g1[D:] / K),
        "Wc_td": kron4(W_cnf[:D] * DT_STEP),
    }
    wc = np.zeros((128, WC_COLS), f32)
    for name in _WSLOTS:
        wc[:, _wslot(name):_wslot(name) + 128] = wparts[name]
    for g in range(4):
        wc[32 * g:32 * (g + 1), EX_G2 + 3 * g:EX_G2 + 3 * (g + 1)] = W_g2
    # ones12: [12, 12] contraction rows (g,e') -> out (g,e): 1 if same g
    for g in range(4):
        for e1 in range(3):
            for e2 in range(3):
                wc[3 * g + e1, EX_ONES + 3 * g + e2] = 1.0
    for e in range(3):
        for g in range(4):
            wc[3 * g + e, EX_SCAT + 128 * e + 32 * g:
               EX_SCAT + 128 * e + 32 * (g + 1)] = 1.0
    wc = wc.astype(bf16)

    eye = np.eye(128, dtype=f32)
    wdr = np.stack([eye, eye], axis=1).astype(f8e4)  # [128, 2, 128]
    return wc, wdr.reshape(128, 2 * 128)


def stage_inputs(inputs):
    """Returns (in_maps, widths, perm)."""
    f32 = np.float32
    cs = np.asarray(inputs["current_state"], f32)
    nb = np.asarray(inputs["neighbor_states"], f32)
    tiers = np.asarray(inputs["tier_ids"], np.int32)

    cnt = np.stack([(tiers == t).sum(-1) for t in range(3)], axis=1)  # [B,3]
    snake_c0 = np.where(cnt[:, 1] % 2 == 0, cnt[:, 0], -cnt[:, 0])
    perm = np.lexsort((snake_c0, cnt[:, 1]))         # rank -> cell

    # per-ST band widths (shared by all cores), padded to even, min 2
    wraw = []
    for t in range(3):
        ct = cnt[perm, t].reshape(NT, ST * N_CORES)
        w = np.maximum(2, ct.max(axis=1))
        wraw.append(w + (w % 2))
    # smallest bands at both ends (fast DMA-bound startup, short tail),
    # largest in the middle
    asc = np.argsort(wraw[0] + wraw[1] + wraw[2], kind="stable")
    border = np.concatenate([asc[0::2], asc[1::2][::-1]])
    widths = tuple(tuple(int(x) for x in w[border]) for w in wraw)

    inv = (1.0 / np.maximum(cnt, 1.0)).astype(f32)   # [B, 3]
    wc, wdr = stage_weights(inputs)

    wmax = [max(w) for w in widths]
    orders = []
    for t in range(3):
        order = np.argsort(tiers != t, axis=1, kind="stable")[:, :wmax[t]]
        orders.append(order)

    in_maps = []
    for c in range(N_CORES):
        cells = perm[c::N_CORES]                     # [BS] cell ids
        cello = np.concatenate([cells[b * ST:(b + 1) * ST] for b in border])
        mall = []
        for i in range(NT):
            cell_i = cello[i * ST:(i + 1) * ST]
            for t in range(3):
                w = widths[t][i]
                od = orders[t][cell_i, :w]
                sel = np.take_along_axis(nb[cell_i], od[:, :, None], axis=1)
                msk = np.take_along_axis(tiers[cell_i], od, axis=1) == t
                x = (sel * msk[:, :, None]).astype(f32)
                if t == 1:
                    q = x.astype(f8e4)               # per-slot accuracy
                else:
                    q = _fp8_error_feedback(x)       # sum accuracy
                # blockT with j outermost, fp8 via f32 view for _nb_blockT
                mall.append(_nb_blockT(q.astype(f32)).astype(f8e4))
        ci = np.empty((128, NT * 4 * SC), f32)
        for i in range(NT):
            cell_i = cello[i * ST:(i + 1) * ST]
            ci[:, (4 * i) * SC:(4 * i + 1) * SC] = _to_blockT(cs[cell_i])
            for t in range(3):
                ci[:, (4 * i + 1 + t) * SC:(4 * i + 2 + t) * SC] = \
                    _to_blockT(np.repeat(inv[cell_i, t:t + 1], D, axis=1))
        in_maps.append({
            "ma": np.concatenate(mall, axis=1),
            "ci": ci.astype(bf16),
            "wc": wc, "wdr": wdr,
        })
    return in_maps, widths, perm, border


_PROGRAM_CACHE = {}


def kernel(**inputs):
    from concourse.bass_utils import run_bass_kernel_spmd

    in_maps, widths, perm, border = stage_inputs(inputs)
    if widths not in _PROGRAM_CACHE:
        _PROGRAM_CACHE[widths] = build_program(widths)
    nc = _PROGRAM_CACHE[widths]

    res = run_bass_kernel_spmd(nc, in_maps, core_ids=list(range(N_CORES)))
    out = np.empty((B, D), np.float32)
    for c in range(N_CORES):
        cells = perm[c::N_CORES]
        cello = np.concatenate([cells[b * ST:(b + 1) * ST] for b in border])
        out[cello] = _from_blockT(
            np.asarray(res.results[c]["out"], np.float32))
    return out
